# revision 40
# baseline (speedup 1.0000x reference)
"""Bass/Trainium2 kernel for nn_CoreAttention (NTK causal attention with
linear phi-correction), SPMD over 8 NeuronCores.

Math (per batch b, head h; q,k,v: [n, d]; Z=phi_kv[h]: [d,d]; kk=|phi_k[h]|: [d,1]):
    phi_q  = ELU(q / d**0.25) + 1
    S      = q @ k.T / sqrt(d)
    A      = exp(S) * causal            # max-shift invariant -> use m=0
    num    = A @ v + phi_q @ Z
    den    = A @ ones + phi_q @ kk
    ctx    = num / den

Sharding: batch*head pairs (32) split 4-per-core across 8 cores. No
cross-core communication.

v3 design notes:
  * phi_q computed on HOST and DMA'd in as phiT.
  * exp split across THREE engines: ScalarE runs exact Exp for the
    early/outer q-blocks (whose rows dominate the L2 norm), VectorE runs
    a Schraudolph bit-trick exp (bits = round(S*A + B) as int16 viewed
    as fp16, causal mask folded in via a bias-mask tile that saturates
    masked lanes to 0x8000 = -0.0), and GpSimdE applies the causal tril
    mask for the ScalarE-exp'd diagonal groups.
  * QK matmuls row-tiled 2x via tile_position (contraction is d=64);
    q/k host-duplicated into both 64-row halves.
  * AV matmul emission staggered one group behind exp so the PE never
    waits on the exp engines.
  * division via reciprocal + one broadcast tensor_tensor per q-block.
"""

import math

import numpy as np

import concourse.bacc as bacc
import concourse.mybir as mybir
from concourse.tile import TileContext

SEQ, BS, H, D = 2048, 2, 16, 64
N_CORES = 8
NPAIR = BS * H            # 32 (b,h) pairs
PPC = NPAIR // N_CORES    # 4 pairs per core
P = 128                   # partition tile
NKT = SEQ // P            # 16 k tiles per pair
QB = 512                  # q-block width (one PSUM bank of fp32)
NQB = SEQ // QB           # 4 q blocks
QT_PER_B = QB // P        # 4 q tiles per block
DA = D + 1                # v augmented with ones column

_C = 1.0 / (D ** 0.25)        # phi input scale
_PHI_SCALE = 2.0 ** -7        # keep phi*Z product in fp16 normal range
_EXP_SCALE = 1.0 / math.sqrt(D)

# Schraudolph fp16 exp: bits = round(S * A + B); bits.view(fp16) ~ exp(S/8).
_SCH_A = 1024.0 * math.log2(math.e) / 8.0
_SCH_B = 15.0 * 1024.0 - 60.0
_SCH_B16 = 15296.0            # nearest-fp16 variant for the all-B tile
_SCH_MASKED = -1.0e6          # saturates int16 convert -> 0x8000 -> fp16 -0.0

# Set by test harness only; grading path uses defaults.
TRACE = False
LAST_RESULT = None

_cached_nc = None

# Per-pair group schedule: (qb, g, engine).  ScalarE ('sc') runs exact Exp
# on the error-critical groups (early q-blocks dominate the output L2
# norm); VectorE ('ve') runs the Schraudolph bit-trick exp elsewhere.
# The order interleaves the two engines 1:1 so both work concurrently
# (two q-blocks are kept open at once; PSUM has exactly 2 spare banks),
# which also keeps the PE dense from the very first groups of pair 0.
_SEQ = [(3, 0, 'sc'), (3, 2, 've'), (0, 0, 'sc'), (3, 3, 've'),
        (0, 1, 'sc'), (3, 4, 've'), (1, 0, 'sc'), (3, 5, 've'),
        (1, 1, 'sc'), (3, 6, 've'), (1, 2, 'sc'), (3, 7, 've'),
        (3, 1, 'sc'), (1, 3, 've'), (2, 0, 'sc'), (2, 3, 've'),
        (2, 2, 'sc'), (2, 4, 've'), (2, 1, 'sc'), (2, 5, 've')]
_ENG = {(qb, g): eng for qb, g, eng in _SEQ}
# last group processed for each qb (triggers division + store)
_LAST = {}
for _qb, _g, _ in _SEQ:
    _LAST[_qb] = _g
_FIRST = {}
for _qb, _g, _ in reversed(_SEQ):
    _FIRST[_qb] = _g


def _build_module():
    f16 = mybir.dt.float16
    f32 = mybir.dt.float32
    i16 = mybir.dt.int16
    Exp = mybir.ActivationFunctionType.Exp
    Alu = mybir.AluOpType

    nc = bacc.Bacc("TRN2", target_bir_lowering=False, debug=False)

    d_qk = nc.dram_tensor("qk", [PPC, P, 2, SEQ], f16, kind="ExternalInput")
    d_ph = nc.dram_tensor("ph", [PPC, D, SEQ], f16, kind="ExternalInput")
    d_vp = nc.dram_tensor("vp", [PPC, P, NKT, DA], f16, kind="ExternalInput")
    d_za = nc.dram_tensor("za", [PPC, D, DA], f16, kind="ExternalInput")
    d_bm = nc.dram_tensor("bm", [P, 2, QB], f32, kind="ExternalInput")
    d_tri = nc.dram_tensor("tri", [P, 2, 2 * P], f16, kind="ExternalInput")
    d_out = nc.dram_tensor("out", [PPC, NQB, P, QT_PER_B, D], f16,
                           kind="ExternalOutput")

    with TileContext(nc) as tc:
        with (
            tc.tile_pool(name="const", bufs=1) as constp,
            tc.tile_pool(name="pairbuf", bufs=3) as pairp,
            tc.tile_pool(name="exbuf", bufs=6) as exp_pool,
            tc.tile_pool(name="scps", bufs=3, space="PSUM") as scp,
            tc.tile_pool(name="numps", bufs=2, space="PSUM") as nump,
            tc.tile_pool(name="outbuf", bufs=4) as outp,
        ):
            # all-B bias tile for the plain Schraudolph groups (fp16 so it
            # can double as the PE warm-up operand).  memset on VectorE:
            # its engine preamble finishes ~2us before the DMA path, so
            # the warm-up burst below starts as early as possible.
            bm_b = constp.tile([P, 2, QB], f16)
            nc.vector.memset(bm_b, _SCH_B16)

            # PE clock warm-up: the HAM un-throttles (1.2 -> 2.4 GHz) only
            # after a fully-busy activity window, so run a dense dep-free
            # matmul burst while the first pair streams in.
            wsc = scp.tile([P, 2, QB], f32, tag="sc")
            for w in range(11):
                # varying lhsT matters: a fixed one gets its LDWEIGHTS
                # elided and the stream never un-throttles
                nc.tensor.matmul(
                    out=wsc[:, w % 2, :],
                    lhsT=bm_b[:, w % 2, (w // 2 % 2) * P:
                              (w // 2 % 2) * P + P],
                    rhs=bm_b[:, 0, :],
                    start=True, stop=True,
                )

            # dual-pattern 0/1 mask for both diagonal blocks of a group
            tri_t = constp.tile([P, 2, 2 * P], f16)
            nc.sync.dma_start(out=tri_t, in_=d_tri[:, :, :])

            pair_tiles = {}
            num_tiles = {}

            def load_pair(pair, phase=None):
                """Chunked pair load: separate DMA instructions land on
                separate queues and run in parallel; spreading phases
                avoids a burst that delays the current pair's output DMA.
                Pair 0 front-loads everything the first steps need."""
                if pair >= PPC:
                    return
                if pair not in pair_tiles:
                    qk = pairp.tile([P, 2, SEQ], f16, tag="qk")
                    phiT = pairp.tile([D, SEQ], f16, tag="phiT")
                    vp = pairp.tile([P, NKT, DA], f16, tag="vp")
                    za = pairp.tile([D, DA], f16, tag="za")
                    pair_tiles[pair] = (qk, phiT, vp, za)
                    # chunk order matches what the qb3-first schedule needs
                    nc.sync.dma_start(out=qk[:, :, 3 * QB:SEQ],
                                      in_=d_qk[pair, :, :, 3 * QB:SEQ])
                    nc.sync.dma_start(out=qk[:, :, 0:QB],
                                      in_=d_qk[pair, :, :, 0:QB])
                    if pair == 0:
                        nc.sync.dma_start(out=za, in_=d_za[pair])
                        nc.sync.dma_start(out=phiT, in_=d_ph[pair])
                        nc.sync.dma_start(out=vp, in_=d_vp[pair])
                    return
                qk, phiT, vp, za = pair_tiles[pair]
                if phase == 1:
                    nc.sync.dma_start(out=qk[:, :, QB:2 * QB],
                                      in_=d_qk[pair, :, :, QB:2 * QB])
                    nc.sync.dma_start(out=qk[:, :, 2 * QB:3 * QB],
                                      in_=d_qk[pair, :, :, 2 * QB:3 * QB])
                    if pair != 0:
                        nc.sync.dma_start(out=phiT, in_=d_ph[pair])
                elif phase == 2 and pair != 0:
                    nc.sync.dma_start(out=vp, in_=d_vp[pair])
                    nc.sync.dma_start(out=za, in_=d_za[pair])

            def c0_of(qb, j):
                # causal column restriction within the q-block for k-tile j
                t = j - 4 * qb
                if t >= 1:
                    return t * P
                return 0

            def emit_qk(step, idx=99):
                pair, qb, g = step
                if (qb, g) == _SEQ[0][:2]:
                    load_pair(pair)
                qk, phiT, vp, za = pair_tiles[pair]
                q0 = qb * QB
                sc = scp.tile([P, 2, QB], f32, tag="sc")
                if idx < 8:
                    # dep-free filler into the soon-overwritten bank: keeps
                    # the PE's HAM activity window busy while the exp
                    # pipeline fills, so it doesn't re-throttle to 1.2 GHz
                    for u in range(2):
                        nc.tensor.matmul(
                            out=sc[:, u, :],
                            lhsT=bm_b[:, u, (idx % 2) * P:(idx % 2) * P + P],
                            rhs=bm_b[:, 1, :],
                            start=True, stop=True)
                for u in range(2):
                    j = 2 * g + u
                    c0 = c0_of(qb, j)
                    h = 64 * u
                    nc.tensor.matmul(
                        out=sc[:, u, c0:QB],
                        lhsT=qk[h:h + 64, 1, j * P: (j + 1) * P],
                        rhs=qk[h:h + 64, 0, q0 + c0: q0 + QB],
                        start=True, stop=True,
                        tile_position=(h, 0),
                    )
                return sc

            def emit_exp(step, sc):
                """exp the score group; returns (ex_tile, is_i16)."""
                pair, qb, g = step
                diag = g >= 2 * qb          # group contains diagonal blocks
                t23 = diag and g == 2 * qb + 1
                if _ENG[(qb, g)] == 'sc':
                    ex = exp_pool.tile([P, 2, QB], f16, tag="exf")
                    if t23:
                        nc.scalar.activation(
                            out=ex[:, 0, 2 * P:QB], in_=sc[:, 0, 2 * P:QB],
                            func=Exp, scale=_EXP_SCALE)
                        nc.scalar.activation(
                            out=ex[:, 1, 3 * P:QB], in_=sc[:, 1, 3 * P:QB],
                            func=Exp, scale=_EXP_SCALE)
                    else:
                        nc.scalar.activation(
                            out=ex[:, :, :], in_=sc[:, :, :],
                            func=Exp, scale=_EXP_SCALE)
                    if diag:
                        # causal tril mask on both diagonal blocks in ONE
                        # GpSimd op (otherwise-idle engine; its semaphore
                        # handling is slow, so batching matters): the
                        # [tril|1]/[1|tril] dual-pattern tile covers the
                        # 256-column span holding both blocks.
                        off = 2 * P if t23 else 0
                        blk = ex[:, :, off:off + 2 * P]
                        nc.gpsimd.tensor_mul(out=blk, in0=blk,
                                             in1=tri_t[:, :, :])
                    return ex, False
                ex = exp_pool.tile([P, 2, QB], i16, tag="exi")
                if not diag:
                    nc.vector.scalar_tensor_tensor(
                        out=ex[:, :, :], in0=sc[:, :, :],
                        scalar=_SCH_A, in1=bm_b[:, :, :],
                        op0=Alu.mult, op1=Alu.add)
                elif not t23:
                    nc.vector.scalar_tensor_tensor(
                        out=ex[:, :, :], in0=sc[:, :, :],
                        scalar=_SCH_A, in1=bm_t[:, :, :],
                        op0=Alu.mult, op1=Alu.add)
                else:
                    # bm_t[:, 0, 0:128] is the triangular pattern and
                    # bm_t[:, 0, 128:256] is all-B, so slices of bm_t
                    # line up with both valid regions.
                    nc.vector.scalar_tensor_tensor(
                        out=ex[:, 0, 2 * P:QB], in0=sc[:, 0, 2 * P:QB],
                        scalar=_SCH_A, in1=bm_t[:, 0, 0:2 * P],
                        op0=Alu.mult, op1=Alu.add)
                    nc.vector.scalar_tensor_tensor(
                        out=ex[:, 1, 3 * P:QB], in0=sc[:, 1, 3 * P:QB],
                        scalar=_SCH_A, in1=bm_t[:, 0, 0:P],
                        op0=Alu.mult, op1=Alu.add)
                return ex, True

            def emit_av(unit):
                """AV matmuls for one exp'd group, plus the q-block
                finalization (division + store) after its last group."""
                pair, qb, g, ex, is_i16 = unit
                vp = pair_tiles[pair][2]
                q0 = qb * QB
                num_t = num_tiles[(pair, qb)]
                is_last = g == _LAST[qb]
                for u in range(2):
                    j = 2 * g + u
                    t = j - 4 * qb
                    for qt in range(max(0, t), QT_PER_B):
                        last = (is_last and u == 1 and qt == QT_PER_B - 1)
                        lhsT = ex[:, u, qt * P: (qt + 1) * P]
                        if is_i16:
                            lhsT = lhsT.bitcast(f16)
                        nc.tensor.matmul(
                            out=num_t[:, qt, :],
                            lhsT=lhsT,
                            rhs=vp[:, j, :],
                            start=False, stop=last)
                if is_last:
                    num_tiles.pop((pair, qb))
                    rec = outp.tile([P, QT_PER_B, 1], f32, tag="rec")
                    nc.vector.reciprocal(out=rec, in_=num_t[:, :, D:DA])
                    out_t = outp.tile([P, QT_PER_B, D], f16, tag="out_t")
                    nc.vector.tensor_mul(
                        out=out_t[:, :, :],
                        in0=num_t[:, :, 0:D],
                        in1=rec[:, :, :].broadcast_to([P, QT_PER_B, D]))
                    nc.sync.dma_start(out=d_out[pair, qb], in_=out_t)

            # pair-0 criticals land before the (large) bias-mask consts
            load_pair(0)
            bm_t = constp.tile([P, 2, QB], f32)
            nc.sync.dma_start(out=bm_t, in_=d_bm[:, :, :])
            load_pair(0, 1)

            steps = [(pair, qb, g)
                     for pair in range(PPC)
                     for qb, g, _eng in _SEQ]
            LOOKAHEAD = 2
            sc_tiles = {}
            for i in range(min(LOOKAHEAD, len(steps))):
                sc_tiles[i] = emit_qk(steps[i], i)

            # AV emission staggered behind exp so the PE's in-order queue
            # never blocks on an exp (or GpSimd mask) still in flight.
            # ScalarE diagonal groups go through the GpSimd mask and need
            # an extra slot of slack.
            pending = []   # (emit_at, unit)
            for i, step in enumerate(steps):
                if i + LOOKAHEAD < len(steps):
                    sc_tiles[i + LOOKAHEAD] = emit_qk(
                        steps[i + LOOKAHEAD], i + LOOKAHEAD)
                pair, qb, g = step
                ph_i = i % len(_SEQ)
                if ph_i in (3, 6, 9):
                    load_pair(pair + 1, (None, 1, 2)[(ph_i - 3) // 3])
                qk, phiT, vp, za = pair_tiles[pair]
                q0 = qb * QB
                sc = sc_tiles.pop(i)

                ex, is_i16 = emit_exp(step, sc)

                # drain due AV work BEFORE opening a new q-block, so the
                # PE-queue order (prev qb's last AV -> division -> new phi)
                # can never deadlock on the num-pool buffer handoff
                while pending and pending[0][0] <= i:
                    emit_av(pending.pop(0)[1])

                if g == _FIRST[qb]:
                    # open the num accumulation group: phi_q @ [Z|kk]
                    num_t = nump.tile([P, QT_PER_B, DA], f32, tag="num")
                    num_tiles[(pair, qb)] = num_t
                    for qt in range(QT_PER_B):
                        nc.tensor.matmul(
                            out=num_t[:, qt, :],
                            lhsT=phiT[:, q0 + qt * P: q0 + (qt + 1) * P],
                            rhs=za,
                            start=(qt == 0), stop=False)

                delay = 3 if (_ENG[(qb, g)] == 'sc' and g >= 2 * qb) else 1
                if g == _LAST[qb]:
                    # the q-block's division must be emitted before the
                    # next phi-open can need its num buffer (PE is FIFO)
                    delay = min(delay, 2)
                pending.append((i + delay, (pair, qb, g, ex, is_i16)))
            for _, unit in pending:
                emit_av(unit)

    nc.compile()
    return nc


def _prep_core_inputs(query_layer, key_layer, value_layer, phi_k, phi_kv):
    q = np.asarray(query_layer, dtype=np.float32)
    k = np.asarray(key_layer, dtype=np.float32)
    v = np.asarray(value_layer, dtype=np.float32)
    zk = np.abs(np.asarray(phi_k, dtype=np.float32))[0, :, :, 0]   # [H, D]
    zv = np.asarray(phi_kv, dtype=np.float32)[0]                   # [H, D, D]

    # [seq,bs,h,d] -> per-pair transposed [pair, d, seq]
    qT = np.ascontiguousarray(q.transpose(1, 2, 3, 0).reshape(NPAIR, D, SEQ))
    kT = np.ascontiguousarray(k.transpose(1, 2, 3, 0).reshape(NPAIR, D, SEQ))

    # interleave q/k and duplicate into both 64-row halves for PE row tiling
    qkt = np.stack([qT, kT], axis=2)                # [pair, 64, 2, seq]
    qk2 = np.concatenate([qkt, qkt], axis=1)        # [pair, 128, 2, seq]

    # host phi: ELU(q*_C) + 1
    xs = qT * _C
    ph = np.where(xs > 0.0, xs + 1.0, np.exp(np.minimum(xs, 0.0)))
    ph = ph * _PHI_SCALE                            # [pair, 64, seq]

    vn = v.transpose(1, 2, 0, 3).reshape(NPAIR, SEQ, D)            # [pair, n, d]
    v_aug = np.concatenate(
        [vn, np.ones((NPAIR, SEQ, 1), np.float32)], axis=2)        # [pair, n, 65]
    vp = np.ascontiguousarray(
        v_aug.reshape(NPAIR, NKT, P, DA).transpose(0, 2, 1, 3))    # [pair, p, j, 65]

    za_h = np.concatenate([zv, zk[:, :, None]], axis=2) / _PHI_SCALE  # [H, D, 65]
    za = za_h[np.arange(NPAIR) % H]                                # [pair, d, 65]

    # bias-mask tile for the VectorE Schraudolph exp
    tri = np.where(np.arange(P)[None, :] >= np.arange(P)[:, None],
                   _SCH_B, _SCH_MASKED).astype(np.float32)         # [k, q]
    bm = np.full((P, 2, QB), _SCH_B, np.float32)
    bm[:, 0, 0:P] = tri
    bm[:, 1, 0:P] = _SCH_MASKED
    bm[:, 1, P:2 * P] = tri

    # dual-pattern 0/1 mask for GpSimd diagonal-block masking: within a
    # (t0,t1) or (t2,t3) group's 256-col span, u=0 has its diag block in
    # the first 128 cols and u=1 in the second 128.
    t01 = np.triu(np.ones((P, P), np.float32))
    tri2 = np.ones((P, 2, 2 * P), np.float32)
    tri2[:, 0, 0:P] = t01
    tri2[:, 1, P:2 * P] = t01

    in_maps = []
    for c in range(N_CORES):
        s = slice(c * PPC, (c + 1) * PPC)
        in_maps.append({
            "qk": qk2[s].astype(np.float16),
            "ph": ph[s].astype(np.float16),
            "vp": vp[s].astype(np.float16),
            "za": za[s].astype(np.float16),
            "bm": bm,
            "tri": tri2.astype(np.float16),
        })
    return in_maps


def _install_trace_shim():
    import sys
    import types
    if "antenv.axon_hooks" not in sys.modules:
        m = types.ModuleType("antenv.axon_hooks")
        m._hook = None
        m.set_axon_ntff_profile_hook = lambda h: setattr(m, "_hook", h)
        m.get_axon_ntff_profile_hook = lambda: m._hook
        sys.modules["antenv.axon_hooks"] = m
        import antenv
        antenv.axon_hooks = m
    from trn_agent_boot.trn_boot import _ntff_profile_via_ctypes
    sys.modules["antenv.axon_hooks"].set_axon_ntff_profile_hook(
        _ntff_profile_via_ctypes("/opt/axon/libaxon_pjrt.so"))
    import concourse.bass_utils as bu
    bu.upload_artifacts = lambda tmpdir: "local://" + str(tmpdir)


def kernel(query_layer, key_layer, value_layer, attention_mask, phi_k, phi_kv):
    global _cached_nc, LAST_RESULT
    from concourse.bass_utils import run_bass_kernel_spmd

    if TRACE:
        _install_trace_shim()
    if _cached_nc is None:
        _cached_nc = _build_module()
    nc = _cached_nc

    in_maps = _prep_core_inputs(
        query_layer, key_layer, value_layer, phi_k, phi_kv)
    res = run_bass_kernel_spmd(
        nc, in_maps, core_ids=list(range(N_CORES)), trace=TRACE)
    LAST_RESULT = res

    outs = np.stack([res.results[c]["out"] for c in range(N_CORES)])
    # [8, PPC, NQB, P, QT, D] -> row q = qb*512 + qt*128 + p
    outs = outs.reshape(NPAIR, NQB, P, QT_PER_B, D)
    ctx = outs.transpose(0, 1, 3, 2, 4).reshape(BS, H, SEQ, D)
    ctx = ctx.transpose(2, 0, 1, 3)                               # [n,bs,h,d]
    return np.ascontiguousarray(ctx.reshape(SEQ, BS, H * D)).astype(np.float32)


# revision 42
# speedup vs baseline: 1.0268x; 1.0268x over previous
"""Bass/Trainium2 kernel for nn_CoreAttention (NTK causal attention with
linear phi-correction), SPMD over 8 NeuronCores.

Math (per batch b, head h; q,k,v: [n, d]; Z=phi_kv[h]: [d,d]; kk=|phi_k[h]|: [d,1]):
    phi_q  = ELU(q / d**0.25) + 1
    S      = q @ k.T / sqrt(d)
    A      = exp(S) * causal            # max-shift invariant -> use m=0
    num    = A @ v + phi_q @ Z
    den    = A @ ones + phi_q @ kk
    ctx    = num / den

Sharding: batch*head pairs (32) split 4-per-core across 8 cores. No
cross-core communication.

v3 design notes:
  * phi_q computed on HOST and DMA'd in as phiT.
  * exp split across THREE engines: ScalarE runs exact Exp for the
    early/outer q-blocks (whose rows dominate the L2 norm), VectorE runs
    a Schraudolph bit-trick exp (bits = round(S*A + B) as int16 viewed
    as fp16, causal mask folded in via a bias-mask tile that saturates
    masked lanes to 0x8000 = -0.0), and GpSimdE applies the causal tril
    mask for the ScalarE-exp'd diagonal groups.
  * QK matmuls row-tiled 2x via tile_position (contraction is d=64);
    q/k host-duplicated into both 64-row halves.
  * AV matmul emission staggered one group behind exp so the PE never
    waits on the exp engines.
  * division via reciprocal + one broadcast tensor_tensor per q-block.
"""

import math

import numpy as np

import concourse.bacc as bacc
import concourse.mybir as mybir
from concourse.tile import TileContext

SEQ, BS, H, D = 2048, 2, 16, 64
N_CORES = 8
NPAIR = BS * H            # 32 (b,h) pairs
PPC = NPAIR // N_CORES    # 4 pairs per core
P = 128                   # partition tile
NKT = SEQ // P            # 16 k tiles per pair
QB = 512                  # q-block width (one PSUM bank of fp32)
NQB = SEQ // QB           # 4 q blocks
QT_PER_B = QB // P        # 4 q tiles per block
DA = D + 1                # v augmented with ones column

_C = 1.0 / (D ** 0.25)        # phi input scale
_PHI_SCALE = 2.0 ** -7        # keep phi*Z product in fp16 normal range
_EXP_SCALE = 1.0 / math.sqrt(D)

# Schraudolph fp16 exp: bits = round(S * A + B); bits.view(fp16) ~ exp(S/8).
_SCH_A = 1024.0 * math.log2(math.e) / 8.0
_SCH_B = 15.0 * 1024.0 - 60.0
_SCH_B16 = 15296.0            # nearest-fp16 variant for the all-B tile
_SCH_MASKED = -1.0e6          # saturates int16 convert -> 0x8000 -> fp16 -0.0

# Set by test harness only; grading path uses defaults.
TRACE = False
LAST_RESULT = None

_cached_nc = None

# Per-pair group schedule: (qb, g, engine).  ScalarE ('sc') runs exact Exp
# on the error-critical groups (early q-blocks dominate the output L2
# norm); VectorE ('ve') runs the Schraudolph bit-trick exp elsewhere.
# The order interleaves the two engines 1:1 so both work concurrently
# (two q-blocks are kept open at once; PSUM has exactly 2 spare banks),
# which also keeps the PE dense from the very first groups of pair 0.
_SEQ = [(3, 0, 'sc'), (3, 2, 've'), (0, 0, 'sc'), (3, 3, 've'),
        (0, 1, 'sc'), (3, 4, 've'), (1, 0, 'sc'), (3, 5, 've'),
        (1, 1, 'sc'), (3, 6, 've'), (1, 2, 'sc'), (3, 7, 've'),
        (3, 1, 'sc'), (1, 3, 've'), (2, 0, 'sc'), (2, 3, 've'),
        (2, 2, 'sc'), (2, 4, 've'), (2, 1, 'sc'), (2, 5, 've')]
_ENG = {(qb, g): eng for qb, g, eng in _SEQ}
# last group processed for each qb (triggers division + store)
_LAST = {}
for _qb, _g, _ in _SEQ:
    _LAST[_qb] = _g
_FIRST = {}
for _qb, _g, _ in reversed(_SEQ):
    _FIRST[_qb] = _g


def _build_module():
    f16 = mybir.dt.float16
    f32 = mybir.dt.float32
    i16 = mybir.dt.int16
    Exp = mybir.ActivationFunctionType.Exp
    Alu = mybir.AluOpType

    nc = bacc.Bacc("TRN2", target_bir_lowering=False, debug=False)

    d_qk = nc.dram_tensor("qk", [PPC, P, 2, SEQ], f16, kind="ExternalInput")
    d_ph = nc.dram_tensor("ph", [PPC, D, SEQ], f16, kind="ExternalInput")
    d_vp = nc.dram_tensor("vp", [PPC, P, NKT, DA], f16, kind="ExternalInput")
    d_za = nc.dram_tensor("za", [PPC, D, DA], f16, kind="ExternalInput")
    d_bm = nc.dram_tensor("bm", [P, 2, QB], f32, kind="ExternalInput")
    d_tri = nc.dram_tensor("tri", [P, 2, 2 * P], f16, kind="ExternalInput")
    d_out = nc.dram_tensor("out", [PPC, NQB, P, QT_PER_B, D], f16,
                           kind="ExternalOutput")

    with TileContext(nc) as tc:
        with (
            tc.tile_pool(name="const", bufs=1) as constp,
            tc.tile_pool(name="pairbuf", bufs=3) as pairp,
            tc.tile_pool(name="exbuf", bufs=6) as exp_pool,
            tc.tile_pool(name="scps", bufs=3, space="PSUM") as scp,
            tc.tile_pool(name="numps", bufs=2, space="PSUM") as nump,
            tc.tile_pool(name="outbuf", bufs=4) as outp,
        ):
            # all-B bias tile for the plain Schraudolph groups (fp16 so it
            # can double as the PE warm-up operand).  memset on VectorE:
            # its engine preamble finishes ~2us before the DMA path, so
            # the warm-up burst below starts as early as possible.
            bm_b = constp.tile([P, 2, QB], f16)
            nc.vector.memset(bm_b, _SCH_B16)

            # PE clock warm-up: the HAM un-throttles (1.2 -> 2.4 GHz) only
            # after a fully-busy activity window, so run a dense dep-free
            # matmul burst while the first pair streams in.
            wsc = scp.tile([P, 2, QB], f32, tag="sc")
            for w in range(11):
                # varying lhsT matters: a fixed one gets its LDWEIGHTS
                # elided and the stream never un-throttles
                nc.tensor.matmul(
                    out=wsc[:, w % 2, :],
                    lhsT=bm_b[:, w % 2, (w // 2 % 2) * P:
                              (w // 2 % 2) * P + P],
                    rhs=bm_b[:, 0, :],
                    start=True, stop=True,
                )

            # dual-pattern 0/1 mask for both diagonal blocks of a group
            tri_t = constp.tile([P, 2, 2 * P], f16)
            nc.sync.dma_start(out=tri_t, in_=d_tri[:, :, :])

            pair_tiles = {}
            num_tiles = {}

            def load_pair(pair, phase=None):
                """Chunked pair load: separate DMA instructions land on
                separate queues and run in parallel; spreading phases
                avoids a burst that delays the current pair's output DMA.
                Pair 0 front-loads everything the first steps need."""
                if pair >= PPC:
                    return
                if pair not in pair_tiles:
                    qk = pairp.tile([P, 2, SEQ], f16, tag="qk")
                    phiT = pairp.tile([D, SEQ], f16, tag="phiT")
                    vp = pairp.tile([P, NKT, DA], f16, tag="vp")
                    za = pairp.tile([D, DA], f16, tag="za")
                    pair_tiles[pair] = (qk, phiT, vp, za)
                    # chunk order matches what the qb3-first schedule needs
                    nc.sync.dma_start(out=qk[:, :, 3 * QB:SEQ],
                                      in_=d_qk[pair, :, :, 3 * QB:SEQ])
                    nc.sync.dma_start(out=qk[:, :, 0:QB],
                                      in_=d_qk[pair, :, :, 0:QB])
                    if pair == 0:
                        nc.sync.dma_start(out=za, in_=d_za[pair])
                        nc.sync.dma_start(out=phiT, in_=d_ph[pair])
                        nc.sync.dma_start(out=vp, in_=d_vp[pair])
                    return
                qk, phiT, vp, za = pair_tiles[pair]
                if phase == 1:
                    nc.sync.dma_start(out=qk[:, :, QB:2 * QB],
                                      in_=d_qk[pair, :, :, QB:2 * QB])
                    nc.sync.dma_start(out=qk[:, :, 2 * QB:3 * QB],
                                      in_=d_qk[pair, :, :, 2 * QB:3 * QB])
                    if pair != 0:
                        nc.sync.dma_start(out=phiT, in_=d_ph[pair])
                elif phase == 2 and pair != 0:
                    nc.sync.dma_start(out=vp, in_=d_vp[pair])
                    nc.sync.dma_start(out=za, in_=d_za[pair])

            def c0_of(qb, j):
                # causal column restriction within the q-block for k-tile j
                t = j - 4 * qb
                if t >= 1:
                    return t * P
                return 0

            def emit_qk(step, idx=99):
                pair, qb, g = step
                if (qb, g) == _SEQ[0][:2]:
                    load_pair(pair)
                qk, phiT, vp, za = pair_tiles[pair]
                q0 = qb * QB
                sc = scp.tile([P, 2, QB], f32, tag="sc")
                if idx < 8:
                    # dep-free filler into the soon-overwritten bank: keeps
                    # the PE's HAM activity window busy while the exp
                    # pipeline fills, so it doesn't re-throttle to 1.2 GHz
                    for u in range(2):
                        nc.tensor.matmul(
                            out=sc[:, u, :],
                            lhsT=bm_b[:, u, (idx % 2) * P:(idx % 2) * P + P],
                            rhs=bm_b[:, 1, :],
                            start=True, stop=True)
                for u in range(2):
                    j = 2 * g + u
                    c0 = c0_of(qb, j)
                    h = 64 * u
                    nc.tensor.matmul(
                        out=sc[:, u, c0:QB],
                        lhsT=qk[h:h + 64, 1, j * P: (j + 1) * P],
                        rhs=qk[h:h + 64, 0, q0 + c0: q0 + QB],
                        start=True, stop=True,
                        tile_position=(h, 0),
                    )
                return sc

            def emit_exp(step, sc, fill=False):
                """exp the score group; returns (ex_tile, is_i16)."""
                pair, qb, g = step
                diag = g >= 2 * qb          # group contains diagonal blocks
                t23 = diag and g == 2 * qb + 1
                if fill:
                    # pipeline-fill steps of pair 0: no cross-group engine
                    # parallelism exists yet, so split THIS group across
                    # both engines (u=0 exact on ScalarE, u=1 Schraudolph
                    # on VectorE) to halve the group's exp latency.
                    ex = exp_pool.tile([P, 2, QB], i16, tag="exi")
                    if t23:
                        nc.scalar.activation(
                            out=ex[:, 0, 2 * P:QB].bitcast(f16),
                            in_=sc[:, 0, 2 * P:QB],
                            func=Exp, scale=_EXP_SCALE)
                        blk = ex[:, 0, 2 * P:3 * P].bitcast(f16)
                        nc.gpsimd.tensor_mul(out=blk, in0=blk,
                                             in1=tri_t[:, 0, 0:P])
                        nc.vector.scalar_tensor_tensor(
                            out=ex[:, 1, 3 * P:QB], in0=sc[:, 1, 3 * P:QB],
                            scalar=_SCH_A, in1=bm_t[:, 0, 0:P],
                            op0=Alu.mult, op1=Alu.add)
                    else:
                        nc.scalar.activation(
                            out=ex[:, 0, :].bitcast(f16), in_=sc[:, 0, :],
                            func=Exp, scale=_EXP_SCALE)
                        if diag:   # t0 diag block in u=0
                            blk = ex[:, 0, 0:P].bitcast(f16)
                            nc.gpsimd.tensor_mul(out=blk, in0=blk,
                                                 in1=tri_t[:, 0, 0:P])
                        nc.vector.scalar_tensor_tensor(
                            out=ex[:, 1, :], in0=sc[:, 1, :],
                            scalar=_SCH_A,
                            in1=bm_t[:, 1, :] if diag else bm_b[:, 1, :],
                            op0=Alu.mult, op1=Alu.add)
                    return ex, True
                if _ENG[(qb, g)] == 'sc':
                    ex = exp_pool.tile([P, 2, QB], f16, tag="exf")
                    if t23:
                        nc.scalar.activation(
                            out=ex[:, 0, 2 * P:QB], in_=sc[:, 0, 2 * P:QB],
                            func=Exp, scale=_EXP_SCALE)
                        nc.scalar.activation(
                            out=ex[:, 1, 3 * P:QB], in_=sc[:, 1, 3 * P:QB],
                            func=Exp, scale=_EXP_SCALE)
                    else:
                        nc.scalar.activation(
                            out=ex[:, :, :], in_=sc[:, :, :],
                            func=Exp, scale=_EXP_SCALE)
                    if diag:
                        # causal tril mask on both diagonal blocks in ONE
                        # GpSimd op (otherwise-idle engine; its semaphore
                        # handling is slow, so batching matters): the
                        # [tril|1]/[1|tril] dual-pattern tile covers the
                        # 256-column span holding both blocks.
                        off = 2 * P if t23 else 0
                        blk = ex[:, :, off:off + 2 * P]
                        nc.gpsimd.tensor_mul(out=blk, in0=blk,
                                             in1=tri_t[:, :, :])
                    return ex, False
                ex = exp_pool.tile([P, 2, QB], i16, tag="exi")
                if not diag:
                    nc.vector.scalar_tensor_tensor(
                        out=ex[:, :, :], in0=sc[:, :, :],
                        scalar=_SCH_A, in1=bm_b[:, :, :],
                        op0=Alu.mult, op1=Alu.add)
                elif not t23:
                    nc.vector.scalar_tensor_tensor(
                        out=ex[:, :, :], in0=sc[:, :, :],
                        scalar=_SCH_A, in1=bm_t[:, :, :],
                        op0=Alu.mult, op1=Alu.add)
                else:
                    # bm_t[:, 0, 0:128] is the triangular pattern and
                    # bm_t[:, 0, 128:256] is all-B, so slices of bm_t
                    # line up with both valid regions.
                    nc.vector.scalar_tensor_tensor(
                        out=ex[:, 0, 2 * P:QB], in0=sc[:, 0, 2 * P:QB],
                        scalar=_SCH_A, in1=bm_t[:, 0, 0:2 * P],
                        op0=Alu.mult, op1=Alu.add)
                    nc.vector.scalar_tensor_tensor(
                        out=ex[:, 1, 3 * P:QB], in0=sc[:, 1, 3 * P:QB],
                        scalar=_SCH_A, in1=bm_t[:, 0, 0:P],
                        op0=Alu.mult, op1=Alu.add)
                return ex, True

            def emit_av(unit):
                """AV matmuls for one exp'd group, plus the q-block
                finalization (division + store) after its last group."""
                pair, qb, g, ex, is_i16 = unit
                vp = pair_tiles[pair][2]
                q0 = qb * QB
                num_t = num_tiles[(pair, qb)]
                is_last = g == _LAST[qb]
                for u in range(2):
                    j = 2 * g + u
                    t = j - 4 * qb
                    for qt in range(max(0, t), QT_PER_B):
                        last = (is_last and u == 1 and qt == QT_PER_B - 1)
                        lhsT = ex[:, u, qt * P: (qt + 1) * P]
                        if is_i16:
                            lhsT = lhsT.bitcast(f16)
                        nc.tensor.matmul(
                            out=num_t[:, qt, :],
                            lhsT=lhsT,
                            rhs=vp[:, j, :],
                            start=False, stop=last)
                if is_last:
                    num_tiles.pop((pair, qb))
                    rec = outp.tile([P, QT_PER_B, 1], f32, tag="rec")
                    nc.vector.reciprocal(out=rec, in_=num_t[:, :, D:DA])
                    out_t = outp.tile([P, QT_PER_B, D], f16, tag="out_t")
                    nc.vector.tensor_mul(
                        out=out_t[:, :, :],
                        in0=num_t[:, :, 0:D],
                        in1=rec[:, :, :].broadcast_to([P, QT_PER_B, D]))
                    nc.sync.dma_start(out=d_out[pair, qb], in_=out_t)

            # pair-0 criticals land before the (large) bias-mask consts
            load_pair(0)
            bm_t = constp.tile([P, 2, QB], f32)
            nc.sync.dma_start(out=bm_t, in_=d_bm[:, :, :])
            load_pair(0, 1)

            steps = [(pair, qb, g)
                     for pair in range(PPC)
                     for qb, g, _eng in _SEQ]
            LOOKAHEAD = 2
            sc_tiles = {}
            for i in range(min(LOOKAHEAD, len(steps))):
                sc_tiles[i] = emit_qk(steps[i], i)

            # AV emission staggered behind exp so the PE's in-order queue
            # never blocks on an exp (or GpSimd mask) still in flight.
            # ScalarE diagonal groups go through the GpSimd mask and need
            # an extra slot of slack.
            pending = []   # (emit_at, unit)
            for i, step in enumerate(steps):
                if i + LOOKAHEAD < len(steps):
                    sc_tiles[i + LOOKAHEAD] = emit_qk(
                        steps[i + LOOKAHEAD], i + LOOKAHEAD)
                pair, qb, g = step
                ph_i = i % len(_SEQ)
                if ph_i in (3, 6, 9):
                    load_pair(pair + 1, (None, 1, 2)[(ph_i - 3) // 3])
                qk, phiT, vp, za = pair_tiles[pair]
                q0 = qb * QB
                sc = sc_tiles.pop(i)

                ex, is_i16 = emit_exp(step, sc, fill=(i < 6))

                # drain due AV work BEFORE opening a new q-block, so the
                # PE-queue order (prev qb's last AV -> division -> new phi)
                # can never deadlock on the num-pool buffer handoff
                while pending and pending[0][0] <= i:
                    emit_av(pending.pop(0)[1])

                if g == _FIRST[qb]:
                    # open the num accumulation group: phi_q @ [Z|kk]
                    num_t = nump.tile([P, QT_PER_B, DA], f32, tag="num")
                    num_tiles[(pair, qb)] = num_t
                    for qt in range(QT_PER_B):
                        nc.tensor.matmul(
                            out=num_t[:, qt, :],
                            lhsT=phiT[:, q0 + qt * P: q0 + (qt + 1) * P],
                            rhs=za,
                            start=(qt == 0), stop=False)

                delay = 3 if (_ENG[(qb, g)] == 'sc' and g >= 2 * qb) else 1
                if g == _LAST[qb]:
                    # the q-block's division must be emitted before the
                    # next phi-open can need its num buffer (PE is FIFO)
                    delay = min(delay, 2)
                pending.append((i + delay, (pair, qb, g, ex, is_i16)))
            for _, unit in pending:
                emit_av(unit)

    nc.compile()
    return nc


def _prep_core_inputs(query_layer, key_layer, value_layer, phi_k, phi_kv):
    q = np.asarray(query_layer, dtype=np.float32)
    k = np.asarray(key_layer, dtype=np.float32)
    v = np.asarray(value_layer, dtype=np.float32)
    zk = np.abs(np.asarray(phi_k, dtype=np.float32))[0, :, :, 0]   # [H, D]
    zv = np.asarray(phi_kv, dtype=np.float32)[0]                   # [H, D, D]

    # [seq,bs,h,d] -> per-pair transposed [pair, d, seq]
    qT = np.ascontiguousarray(q.transpose(1, 2, 3, 0).reshape(NPAIR, D, SEQ))
    kT = np.ascontiguousarray(k.transpose(1, 2, 3, 0).reshape(NPAIR, D, SEQ))

    # interleave q/k and duplicate into both 64-row halves for PE row tiling
    qkt = np.stack([qT, kT], axis=2)                # [pair, 64, 2, seq]
    qk2 = np.concatenate([qkt, qkt], axis=1)        # [pair, 128, 2, seq]

    # host phi: ELU(q*_C) + 1
    xs = qT * _C
    ph = np.where(xs > 0.0, xs + 1.0, np.exp(np.minimum(xs, 0.0)))
    ph = ph * _PHI_SCALE                            # [pair, 64, seq]

    vn = v.transpose(1, 2, 0, 3).reshape(NPAIR, SEQ, D)            # [pair, n, d]
    v_aug = np.concatenate(
        [vn, np.ones((NPAIR, SEQ, 1), np.float32)], axis=2)        # [pair, n, 65]
    vp = np.ascontiguousarray(
        v_aug.reshape(NPAIR, NKT, P, DA).transpose(0, 2, 1, 3))    # [pair, p, j, 65]

    za_h = np.concatenate([zv, zk[:, :, None]], axis=2) / _PHI_SCALE  # [H, D, 65]
    za = za_h[np.arange(NPAIR) % H]                                # [pair, d, 65]

    # bias-mask tile for the VectorE Schraudolph exp
    tri = np.where(np.arange(P)[None, :] >= np.arange(P)[:, None],
                   _SCH_B, _SCH_MASKED).astype(np.float32)         # [k, q]
    bm = np.full((P, 2, QB), _SCH_B, np.float32)
    bm[:, 0, 0:P] = tri
    bm[:, 1, 0:P] = _SCH_MASKED
    bm[:, 1, P:2 * P] = tri

    # dual-pattern 0/1 mask for GpSimd diagonal-block masking: within a
    # (t0,t1) or (t2,t3) group's 256-col span, u=0 has its diag block in
    # the first 128 cols and u=1 in the second 128.
    t01 = np.triu(np.ones((P, P), np.float32))
    tri2 = np.ones((P, 2, 2 * P), np.float32)
    tri2[:, 0, 0:P] = t01
    tri2[:, 1, P:2 * P] = t01

    in_maps = []
    for c in range(N_CORES):
        s = slice(c * PPC, (c + 1) * PPC)
        in_maps.append({
            "qk": qk2[s].astype(np.float16),
            "ph": ph[s].astype(np.float16),
            "vp": vp[s].astype(np.float16),
            "za": za[s].astype(np.float16),
            "bm": bm,
            "tri": tri2.astype(np.float16),
        })
    return in_maps


def _install_trace_shim():
    import sys
    import types
    if "antenv.axon_hooks" not in sys.modules:
        m = types.ModuleType("antenv.axon_hooks")
        m._hook = None
        m.set_axon_ntff_profile_hook = lambda h: setattr(m, "_hook", h)
        m.get_axon_ntff_profile_hook = lambda: m._hook
        sys.modules["antenv.axon_hooks"] = m
        import antenv
        antenv.axon_hooks = m
    from trn_agent_boot.trn_boot import _ntff_profile_via_ctypes
    sys.modules["antenv.axon_hooks"].set_axon_ntff_profile_hook(
        _ntff_profile_via_ctypes("/opt/axon/libaxon_pjrt.so"))
    import concourse.bass_utils as bu
    bu.upload_artifacts = lambda tmpdir: "local://" + str(tmpdir)


def kernel(query_layer, key_layer, value_layer, attention_mask, phi_k, phi_kv):
    global _cached_nc, LAST_RESULT
    from concourse.bass_utils import run_bass_kernel_spmd

    if TRACE:
        _install_trace_shim()
    if _cached_nc is None:
        _cached_nc = _build_module()
    nc = _cached_nc

    in_maps = _prep_core_inputs(
        query_layer, key_layer, value_layer, phi_k, phi_kv)
    res = run_bass_kernel_spmd(
        nc, in_maps, core_ids=list(range(N_CORES)), trace=TRACE)
    LAST_RESULT = res

    outs = np.stack([res.results[c]["out"] for c in range(N_CORES)])
    # [8, PPC, NQB, P, QT, D] -> row q = qb*512 + qt*128 + p
    outs = outs.reshape(NPAIR, NQB, P, QT_PER_B, D)
    ctx = outs.transpose(0, 1, 3, 2, 4).reshape(BS, H, SEQ, D)
    ctx = ctx.transpose(2, 0, 1, 3)                               # [n,bs,h,d]
    return np.ascontiguousarray(ctx.reshape(SEQ, BS, H * D)).astype(np.float32)
